# revision 44
# baseline (speedup 1.0000x reference)
"""BlockSparseDilatedAttention TRN2 kernel (v3).

Full inputs q,k,v: [1, 8192, 12, 64] fp32. Output: same shape.

Math: 16 blocks of 512 tokens; block pairs (r, c) with |r-c| <= 2 (74 pairs).
Per pair, dilated segment attention in 3 head-groups of 4 heads:
  g0: seg 128, dil 1 -> 4 units of 128 tokens per block
  g1: seg 256, dil 2 -> 2 units of 128 (odd positions)
  g2: seg 128, dil 4 -> 1 unit of 128 (pos 2 mod 4), block-diag mask of 4x32
Each unit: softmax(Q K^T / 8) V over its own 128 kv tokens, normalized PER
BLOCK PAIR (the reference scatter-adds independently-normalized pair outputs).

Sharding: 8 cores = 2 head-halves x 4 row-quarters. Identical SPMD program;
edge cores get zero-padded kv blocks (zero V => zero contribution).

Device pipeline per sweep (g, u-pair), 4 sweeps = (g0,u01),(g0,u23),(g1,u01),
(g2,u0):
  Scores S^T computed into 7 PSUM tile-groups per sweep (<=384 cols each,
  <=3 banks; groups pack (cb, row-range) entries so every group fits and the
  exp batches 2h x nw x cols in ONE activation instruction -> 28 exps total).
  exp (ScalarE) -> A^T tiles (f16) in SBUF.
  AV per (row, h): 5 deltas x nw matmuls A^T-stationary x [V|1] into a
  [128, nw, 5*66] PSUM tile (per-delta outputs + sums column).
  reciprocal_approx_fast (DVE) of the 5*nw sums; tmp = po * rc (mul);
  out = sum_d tmp (reduce). mul/reduce alternate DVE / Pool engines.
  f16 outputs DMA'd out per row.
DMA: inputs split across the 3 DMA queues (sync+scalar HWDGE, gpsimd SWDGE)
in need-order with per-sweep DRAM layouts; outputs round-robin sync/scalar.
"""

import sys

import numpy as np

for _p in ("/opt/trn_rl_repo",):
    if _p not in sys.path:
        sys.path.append(_p)

# ---------------------------------------------------------------- constants
B, S, H, D = 1, 8192, 12, 64
BLOCK = 512
NB = S // BLOCK            # 16
MASK_M = 512.0
NCORES = 8
ROWS_PER_CORE = 4
CB = 8                     # col blocks per core window (4 rows +-2, padded)
SCALE = 0.125              # 1/sqrt(64)

# sweeps: (group, u-pair)
SWEEPS = [(0, (0, 1)), (0, (2, 3)), (1, (0, 1)), (2, (0,))]

def _rows_for_cb(cb):
    """Local row indices i in [0,3] attending col block cb (c = 4R-2+cb)."""
    return max(0, cb - 4), min(3, cb)


# score chunks (cb, i0, i1) with <=2 rows: each fits a 2-bank PSUM tile
# [128, 2h, 2u, 256] whose four (h,u) sub-regions never cross a bank
SCHUNKS = [(0, 0, 0), (1, 0, 1), (2, 0, 1), (2, 2, 2), (3, 0, 1), (3, 2, 3),
           (4, 0, 1), (4, 2, 3), (5, 1, 2), (5, 3, 3), (6, 2, 3), (7, 3, 3)]
# (i, cb) -> (chunk index, col offset) for the AV stationary slices
AVMAP = {}
for _ci, (_cb, _i0, _i1) in enumerate(SCHUNKS):
    for _i in range(_i0, _i1 + 1):
        AVMAP[(_i, _cb)] = (_ci, (_i - _i0) * 128)
# AV for row i emitted one beat after its last chunk (last chunks: 6,8,10,11)
FIRE = {7: [0], 9: [1], 11: [2], 12: [3]}

# matmul input dtype for Q/K and for A/V
S_DTYPE = "f16"
# exp output / AV stationary: "f16" (rel err ~5e-4) or "f8e4" (halves the AV
# LDWEIGHTS time but measured rel err 1.9e-2 — too close to the 2e-2 gate)
AT_DTYPE = "f16"
# exp(S/8 - 2) keeps exp within e4m3 range (max 240) when AT_DTYPE is f8e4;
# the uniform e^-2 factor cancels in the per-pair softmax normalization
AT_BIAS = -2.0
V_DTYPE = "f16"
# DMA queues: sync (HWDGE) + gpsimd (SWDGE) only. A DMA on an engine's queue
# blocks that engine's instruction stream for the transfer, so the scalar
# queue must stay clear for exp.
QUEUE_NAMES = ("sync", "gpsimd")
# sweeps processed in interleaved pairs so each engine always has the other
# sweep's work while this one's PSUM ring-2 dependencies resolve; the two
# g0 sweeps go first so early AV only waits on v0/v1 DMA
SWEEP_PAIRS = ((0, 1), (2, 3))


def _gather_pos():
    pos = [np.arange(512)]
    pos.append(np.concatenate([s + 1 + 2 * np.arange(128) for s in (0, 256)]))
    pos.append(np.concatenate([s + 2 + 4 * np.arange(32) for s in (0, 128, 256, 384)]))
    return pos


POS = _gather_pos()


def _shapes(s):
    g, up = SWEEPS[s]
    nw = len(up)
    if g < 2:
        kt = (128, CB * nw * 128)
        qt = (128, 4 * nw * 128)
    else:
        kt = (69, 2 * CB * 128)
        qt = (69, 2 * 4 * 128)
    v = (128, 2 * CB * nw * 66)
    out = (128, 2 * 4 * nw * 5 * 64)   # per-delta normalized products; host sums d
    return kt, qt, v, out


# ---------------------------------------------------------------- bass build
_BASS_CACHE = {}


def _build_bass():
    if "nc" in _BASS_CACHE:
        return _BASS_CACHE["nc"]

    import concourse.tile as tile
    from concourse import bacc, mybir

    exp_fn = mybir.ActivationFunctionType.Exp
    AXIS_X = mybir.AxisListType.X
    f32 = mybir.dt.float32
    dt_map = {"f32": mybir.dt.float32, "bf16": mybir.dt.bfloat16,
              "f16": mybir.dt.float16, "f8e4": mybir.dt.float8e4}
    sdt = dt_map[S_DTYPE]
    adt = dt_map[AT_DTYPE]
    vdt = dt_map[V_DTYPE]
    odt = mybir.dt.float16
    nc = bacc.Bacc("TRN2", target_bir_lowering=False, debug=False,
                   enable_asserts=False)
    if AT_DTYPE == "f8e4" and AT_BIAS != 0.0:
        bt = nc.alloc_sbuf_tensor("const-float32-bias", [128, 1], f32)
        nc.gpsimd.memset(bt.ap(), AT_BIAS)
        nc.const_aps.aps[(f32, AT_BIAS)] = bt.ap()
        nc.all_engine_barrier()

    kt_d, qt_d, v_d, out_d = [], [], [], []
    for s in range(4):
        kts, qts, vs, outs = _shapes(s)
        kt_d.append(nc.dram_tensor(f"kt{s}", list(kts), sdt, kind="ExternalInput").ap())
        qt_d.append(nc.dram_tensor(f"qt{s}", list(qts), sdt, kind="ExternalInput").ap())
        v_d.append(nc.dram_tensor(f"v{s}", list(vs), vdt, kind="ExternalInput").ap())
        out_d.append(nc.dram_tensor(f"out{s}", list(outs), odt, kind="ExternalOutput").ap())

    with tile.TileContext(nc) as tc:
        with tc.tile_pool(name="inp", bufs=1) as inp, \
             tc.tile_pool(name="at", bufs=18) as atp, \
             tc.tile_pool(name="small", bufs=8) as small, \
             tc.tile_pool(name="ps_s", bufs=2, space="PSUM") as ps_s, \
             tc.tile_pool(name="ps_o", bufs=2, space="PSUM") as ps_o:

            kt_sb, qt_sb, v_sb = [], [], []
            for s in range(4):
                kts, qts, vs, outs = _shapes(s)
                kt_sb.append(inp.tile(list(kts), sdt, tag=f"kt{s}", name=f"kt{s}"))
                qt_sb.append(inp.tile(list(qts), sdt, tag=f"qt{s}", name=f"qt{s}"))
                v_sb.append(inp.tile(list(vs), vdt, tag=f"v{s}", name=f"v{s}"))

            # ---- input DMAs: need-order chunks round-robin over the queues
            queues = [getattr(nc, qn) for qn in QUEUE_NAMES]
            nq = len(queues)
            chunks = []  # (sbuf_tile, dram_ap, col0, col1)

            def add_chunks(sb, dr, n_split):
                n = dr.shape[1]
                step = (n + n_split - 1) // n_split
                pos = 0
                while pos < n:
                    end = min(n, pos + step)
                    chunks.append((sb, dr, pos, end))
                    pos = end

            for s in (0, 1, 2, 3):
                fine = 4 if s in (0, 1) else 2
                add_chunks(kt_sb[s], kt_d[s], fine)
                add_chunks(qt_sb[s], qt_d[s], max(1, fine // 2))
                add_chunks(v_sb[s], v_d[s], max(1, fine // 2))
            for ci, (sb, dr, c0, c1) in enumerate(chunks):
                q = queues[ci % nq]
                q.dma_start(out=sb[:, c0:c1], in_=dr[:, c0:c1])

            # ---- rearranged views
            kt_r, qt_r, v_r, out_dr = [], [], [], []
            for s in range(4):
                g, up = SWEEPS[s]
                nw = len(up)
                if g < 2:
                    kt_r.append(kt_sb[s].rearrange("p (c w l) -> p c w l", c=CB, w=nw))
                    qt_r.append(qt_sb[s].rearrange("p (i w l) -> p i w l", i=4, w=nw))
                else:
                    kt_r.append(kt_sb[s].rearrange("p (h c l) -> p h c l", h=2, c=CB))
                    qt_r.append(qt_sb[s].rearrange("p (h i l) -> p h i l", h=2, i=4))
                v_r.append(v_sb[s].rearrange("p (h c w x) -> p h c w x",
                                             h=2, c=CB, w=nw))
                out_dr.append(out_d[s].rearrange("p (h i w d x) -> p h i w d x",
                                                 h=2, i=4, w=nw, d=5))

            # ---- sweeps, interleaved in pairs; AV delayed one cb-beat so it
            # reads last beat's exp while this beat's exp streams on ACT
            unit_idx = 0
            at_tiles = {}

            def emit_scores(s, ci):
                g, up = SWEEPS[s]
                nw = len(up)
                cb, i0, i1 = SCHUNKS[ci]
                nr = i1 - i0 + 1
                sp = ps_s.tile([128, 2, 2, 256], f32, tag="s", name="sp")
                for h in range(2):
                    for us, u in enumerate(up):
                        if g < 2:
                            lhsT = kt_r[s][64 * h:64 * h + 64, cb, us, :]
                            rhs = qt_r[s][64 * h:64 * h + 64, i0:i1 + 1, us, :]
                        else:
                            lhsT = kt_r[s][0:69, h, cb, :]
                            rhs = qt_r[s][0:69, h, i0:i1 + 1, :]
                        nc.tensor.matmul(sp[:, h, us, 0:nr * 128], lhsT, rhs,
                                         start=True, stop=True)
                at = atp.tile([128, 2, 2, 256], adt, tag="at", name="at")
                nc.scalar.activation(at[:, :, 0:nw, 0:nr * 128],
                                     sp[:, :, 0:nw, 0:nr * 128],
                                     exp_fn, scale=SCALE,
                                     bias=AT_BIAS if AT_DTYPE == "f8e4" else 0.0)
                at_tiles[(s, ci)] = at

            def emit_av(s, i):
                nonlocal unit_idx
                g, up = SWEEPS[s]
                nw = len(up)
                for h in range(2):
                    po = ps_o.tile([128, 2, 512], f32, tag="o", name="po")
                    for d in range(5):
                        ccb = i + d
                        ci, aoff = AVMAP[(i, ccb)]
                        a_t = at_tiles[(s, ci)]
                        for us, u in enumerate(up):
                            lhsT = a_t[:, h, us, aoff:aoff + 128]
                            rhs = v_r[s][:, h, ccb, us, 0:65]
                            nc.tensor.matmul(po[:, us, d * 66:d * 66 + 65],
                                             lhsT, rhs, start=True, stop=True)
                    pv = po[:, :, 0:330].rearrange("p u (d c) -> p u d c", c=66)
                    rc = small.tile([128, 2, 5], f32, tag="rc", name="rc")
                    nc.vector.reciprocal_approx_fast(
                        out=rc[:, 0:nw], in_=pv[:, 0:nw, 0:5, 64])
                    tmp = small.tile([128, 2, 5, 64], odt, tag="tmp",
                                     name="tmp")
                    nc.vector.tensor_mul(
                        tmp[:, 0:nw], pv[:, 0:nw, 0:5, 0:64],
                        rc[:, 0:nw].broadcast_to([128, nw, 5, 64]))
                    # ship per-delta products; host sums over d
                    queues[unit_idx % nq].dma_start(
                        out=out_dr[s][:, h, i], in_=tmp[:, 0:nw])
                    unit_idx += 1

            nchunk = len(SCHUNKS)
            for pair in SWEEP_PAIRS:
                for t in range(nchunk + 1):
                    for s in pair:
                        if t < nchunk:
                            emit_scores(s, t)
                        for i in FIRE.get(t, []):
                            emit_av(s, i)

    nc.compile()
    _BASS_CACHE["nc"] = nc
    return nc


# ---------------------------------------------------------------- host pack
def _np_dtype(name):
    if name == "bf16":
        import ml_dtypes
        return ml_dtypes.bfloat16
    return np.float16


def _pack_inputs(q, k, v):
    q = np.asarray(q, dtype=np.float32)
    k = np.asarray(k, dtype=np.float32)
    v = np.asarray(v, dtype=np.float32)
    s_np = _np_dtype(S_DTYPE)
    v_np = _np_dtype(V_DTYPE)
    qb = q.reshape(NB, BLOCK, H, D)
    kb = k.reshape(NB, BLOCK, H, D)
    vb = v.reshape(NB, BLOCK, H, D)

    sub = np.repeat(np.arange(4), 32)
    U = (sub[None, :] == np.arange(4)[:, None]).astype(np.float32)  # [4,128]

    in_maps = []
    for core in range(NCORES):
        hh, R = core // 4, core % 4
        m = {}
        for s in range(4):
            g, upair = SWEEPS[s]
            nw = len(upair)
            pos = POS[g]
            if g < 2:
                kt = np.zeros((128, CB, nw, 128), np.float32)
                qt = np.zeros((128, 4, nw, 128), np.float32)
            else:
                kt = np.zeros((69, 2, CB, 128), np.float32)
                qt = np.zeros((69, 2, 4, 128), np.float32)
            va = np.zeros((128, 2, CB, nw, 66), np.float32)
            for h in range(2):
                head = 4 * g + 2 * hh + h
                for i in range(ROWS_PER_CORE):
                    r = 4 * R + i
                    for us, u in enumerate(upair):
                        tok = qb[r, pos[u * 128:(u + 1) * 128], head, :]  # [128,64]
                        if g < 2:
                            qt[64 * h:64 * h + 64, i, us] = tok.T
                        else:
                            qt[0:64, h, i] = tok.T
                            qt[64, h, i] = -MASK_M
                            qt[65:69, h, i] = MASK_M * U
                for cb in range(CB):
                    c = 4 * R - 2 + cb
                    valid = 0 <= c < NB
                    for us, u in enumerate(upair):
                        if valid:
                            tok = kb[c, pos[u * 128:(u + 1) * 128], head, :]
                            vt = vb[c, pos[u * 128:(u + 1) * 128], head, :]
                        else:
                            tok = np.zeros((128, D), np.float32)
                            vt = np.zeros((128, D), np.float32)
                        if g < 2:
                            kt[64 * h:64 * h + 64, cb, us] = tok.T
                        else:
                            kt[0:64, h, cb] = tok.T
                            if valid:
                                kt[64, h, cb] = 1.0
                                kt[65:69, h, cb] = U
                        va[:, h, cb, us, 0:64] = vt
                        va[:, h, cb, us, 64] = 1.0
            m[f"kt{s}"] = kt.reshape(_shapes(s)[0]).astype(s_np)
            m[f"qt{s}"] = qt.reshape(_shapes(s)[1]).astype(s_np)
            m[f"v{s}"] = va.reshape(_shapes(s)[2]).astype(v_np)
        in_maps.append(m)
    return in_maps


def _unpack(results):
    out = np.zeros((B, S, H, D), np.float32)
    for core in range(NCORES):
        hh, R = core // 4, core % 4
        res = results[core]
        for s in range(4):
            g, upair = SWEEPS[s]
            nw = len(upair)
            pos = POS[g]
            og = np.asarray(res[f"out{s}"], dtype=np.float32)
            og = og.reshape(128, 2, 4, nw, 5, 64).sum(axis=4)
            for h in range(2):
                head = 4 * g + 2 * hh + h
                for i in range(ROWS_PER_CORE):
                    r = 4 * R + i
                    for us, u in enumerate(upair):
                        out[0, r * 512 + pos[u * 128:(u + 1) * 128], head, :] = \
                            og[:, h, i, us]
    return out


# ---------------------------------------------------------------- entry
def _run(q, k, v, trace=False):
    from concourse.bass_utils import run_bass_kernel_spmd
    nc = _build_bass()
    in_maps = _pack_inputs(q, k, v)
    res = run_bass_kernel_spmd(nc, in_maps, core_ids=list(range(NCORES)),
                               trace=trace)
    return _unpack(res.results), res


def kernel(q, k, v):
    out, _ = _run(q, k, v, trace=False)
    return out


# revision 48
# speedup vs baseline: 1.1242x; 1.1242x over previous
"""BlockSparseDilatedAttention TRN2 kernel (v3).

Full inputs q,k,v: [1, 8192, 12, 64] fp32. Output: same shape.

Math: 16 blocks of 512 tokens; block pairs (r, c) with |r-c| <= 2 (74 pairs).
Per pair, dilated segment attention in 3 head-groups of 4 heads:
  g0: seg 128, dil 1 -> 4 units of 128 tokens per block
  g1: seg 256, dil 2 -> 2 units of 128 (odd positions)
  g2: seg 128, dil 4 -> 1 unit of 128 (pos 2 mod 4), block-diag mask of 4x32
Each unit: softmax(Q K^T / 8) V over its own 128 kv tokens, normalized PER
BLOCK PAIR (the reference scatter-adds independently-normalized pair outputs).

Sharding: 8 cores = 2 head-halves x 4 row-quarters. Identical SPMD program;
edge cores get zero-padded kv blocks (zero V => zero contribution).

Device pipeline per sweep (g, u-pair), 4 sweeps = (g0,u01),(g0,u23),(g1,u01),
(g2,u0):
  Scores S^T computed into 7 PSUM tile-groups per sweep (<=384 cols each,
  <=3 banks; groups pack (cb, row-range) entries so every group fits and the
  exp batches 2h x nw x cols in ONE activation instruction -> 28 exps total).
  exp (ScalarE) -> A^T tiles (f16) in SBUF.
  AV per (row, h): 5 deltas x nw matmuls A^T-stationary x [V|1] into a
  [128, nw, 5*66] PSUM tile (per-delta outputs + sums column).
  reciprocal_approx_fast (DVE) of the 5*nw sums; tmp = po * rc (mul);
  out = sum_d tmp (reduce). mul/reduce alternate DVE / Pool engines.
  f16 outputs DMA'd out per row.
DMA: inputs split across the 3 DMA queues (sync+scalar HWDGE, gpsimd SWDGE)
in need-order with per-sweep DRAM layouts; outputs round-robin sync/scalar.
"""

import sys

import numpy as np

for _p in ("/opt/trn_rl_repo",):
    if _p not in sys.path:
        sys.path.append(_p)

# ---------------------------------------------------------------- constants
B, S, H, D = 1, 8192, 12, 64
BLOCK = 512
NB = S // BLOCK            # 16
MASK_M = 512.0
NCORES = 8
ROWS_PER_CORE = 4
CB = 8                     # col blocks per core window (4 rows +-2, padded)
SCALE = 0.125              # 1/sqrt(64)

# sweeps: (group, u-pair)
SWEEPS = [(0, (0, 1)), (0, (2, 3)), (1, (0, 1)), (2, (0,))]

def _rows_for_cb(cb):
    """Local row indices i in [0,3] attending col block cb (c = 4R-2+cb)."""
    return max(0, cb - 4), min(3, cb)


# score chunk groups: lists of (cb, i0, i1) entries totalling <=2 rows; each
# group is one 2-bank PSUM tile [128, 2h, 2u, 256] (four bank-legal (h,u)
# sub-regions) and ONE exp — 1-row entries pair up to cut exp count to 40
SCHUNKS = [
    [(0, 0, 0), (2, 2, 2)],
    [(1, 0, 1)],
    [(2, 0, 1)],
    [(3, 0, 1)],
    [(3, 2, 3)],
    [(4, 0, 1)],
    [(4, 2, 3)],
    [(5, 1, 2)],
    [(5, 3, 3), (7, 3, 3)],
    [(6, 2, 3)],
]
# (i, cb) -> (chunk index, col offset) for the AV stationary slices
AVMAP = {}
for _ci, _grp in enumerate(SCHUNKS):
    _off = 0
    for (_cb, _i0, _i1) in _grp:
        for _i in range(_i0, _i1 + 1):
            AVMAP[(_i, _cb)] = (_ci, _off + (_i - _i0) * 128)
        _off += (_i1 - _i0 + 1) * 128
# AV for row i emitted one beat after its last chunk (last chunks: 5,7,9,9)
FIRE = {6: [0], 8: [1], 10: [2, 3]}

# matmul input dtype for Q/K and for A/V
S_DTYPE = "f16"
# exp output / AV stationary: "f16" (rel err ~5e-4) or "f8e4" (halves the AV
# LDWEIGHTS time but measured rel err 1.9e-2 — too close to the 2e-2 gate)
AT_DTYPE = "f16"
# exp(S/8 - 2) keeps exp within e4m3 range (max 240) when AT_DTYPE is f8e4;
# the uniform e^-2 factor cancels in the per-pair softmax normalization
AT_BIAS = -2.0
V_DTYPE = "f16"
# DMA queues: sync (HWDGE) + gpsimd (SWDGE) only. A DMA on an engine's queue
# blocks that engine's instruction stream for the transfer, so the scalar
# queue must stay clear for exp.
QUEUE_NAMES = ("sync", "gpsimd")
# sweeps processed in interleaved pairs so each engine always has the other
# sweep's work while this one's PSUM ring-2 dependencies resolve
SWEEP_PAIRS = ((0, 2), (1, 3))


def _gather_pos():
    pos = [np.arange(512)]
    pos.append(np.concatenate([s + 1 + 2 * np.arange(128) for s in (0, 256)]))
    pos.append(np.concatenate([s + 2 + 4 * np.arange(32) for s in (0, 128, 256, 384)]))
    return pos


POS = _gather_pos()


def _shapes(s):
    g, up = SWEEPS[s]
    nw = len(up)
    if g < 2:
        kt = (128, CB * nw * 128)
        qt = (128, 4 * nw * 128)
    else:
        kt = (69, 2 * CB * 128)
        qt = (69, 2 * 4 * 128)
    v = (128, 2 * CB * nw * 66)
    out = (128, 2 * 4 * nw * 5 * 64)   # per-delta normalized products; host sums d
    return kt, qt, v, out


# ---------------------------------------------------------------- bass build
_BASS_CACHE = {}


def _build_bass():
    if "nc" in _BASS_CACHE:
        return _BASS_CACHE["nc"]

    import concourse.tile as tile
    from concourse import bacc, mybir

    exp_fn = mybir.ActivationFunctionType.Exp
    AXIS_X = mybir.AxisListType.X
    f32 = mybir.dt.float32
    dt_map = {"f32": mybir.dt.float32, "bf16": mybir.dt.bfloat16,
              "f16": mybir.dt.float16, "f8e4": mybir.dt.float8e4}
    sdt = dt_map[S_DTYPE]
    adt = dt_map[AT_DTYPE]
    vdt = dt_map[V_DTYPE]
    odt = mybir.dt.float16
    nc = bacc.Bacc("TRN2", target_bir_lowering=False, debug=False,
                   enable_asserts=False)
    if AT_DTYPE == "f8e4" and AT_BIAS != 0.0:
        bt = nc.alloc_sbuf_tensor("const-float32-bias", [128, 1], f32)
        nc.gpsimd.memset(bt.ap(), AT_BIAS)
        nc.const_aps.aps[(f32, AT_BIAS)] = bt.ap()
        nc.all_engine_barrier()

    kt_d, qt_d, v_d, out_d = [], [], [], []
    for s in range(4):
        kts, qts, vs, outs = _shapes(s)
        kt_d.append(nc.dram_tensor(f"kt{s}", list(kts), sdt, kind="ExternalInput").ap())
        qt_d.append(nc.dram_tensor(f"qt{s}", list(qts), sdt, kind="ExternalInput").ap())
        v_d.append(nc.dram_tensor(f"v{s}", list(vs), vdt, kind="ExternalInput").ap())
        out_d.append(nc.dram_tensor(f"out{s}", list(outs), odt, kind="ExternalOutput").ap())

    with tile.TileContext(nc) as tc:
        with tc.tile_pool(name="inp", bufs=1) as inp, \
             tc.tile_pool(name="at", bufs=18) as atp, \
             tc.tile_pool(name="small", bufs=8) as small, \
             tc.tile_pool(name="ps_s", bufs=2, space="PSUM") as ps_s, \
             tc.tile_pool(name="ps_o", bufs=2, space="PSUM") as ps_o:

            kt_sb, qt_sb, v_sb = [], [], []
            for s in range(4):
                kts, qts, vs, outs = _shapes(s)
                kt_sb.append(inp.tile(list(kts), sdt, tag=f"kt{s}", name=f"kt{s}"))
                qt_sb.append(inp.tile(list(qts), sdt, tag=f"qt{s}", name=f"qt{s}"))
                v_sb.append(inp.tile(list(vs), vdt, tag=f"v{s}", name=f"v{s}"))

            # ---- input DMAs: need-order chunks round-robin over the queues
            queues = [getattr(nc, qn) for qn in QUEUE_NAMES]
            nq = len(queues)
            chunks = []  # (sbuf_tile, dram_ap, col0, col1)

            def add_chunks(sb, dr, n_split):
                n = dr.shape[1]
                step = (n + n_split - 1) // n_split
                pos = 0
                while pos < n:
                    end = min(n, pos + step)
                    chunks.append((sb, dr, pos, end))
                    pos = end

            for s in (0, 2, 1, 3):
                fine = 4 if s in (0, 2) else 2
                add_chunks(kt_sb[s], kt_d[s], fine)
                add_chunks(qt_sb[s], qt_d[s], max(1, fine // 2))
                add_chunks(v_sb[s], v_d[s], max(1, fine // 2))
            for ci, (sb, dr, c0, c1) in enumerate(chunks):
                q = queues[ci % nq]
                q.dma_start(out=sb[:, c0:c1], in_=dr[:, c0:c1])

            # ---- rearranged views
            kt_r, qt_r, v_r, out_dr = [], [], [], []
            for s in range(4):
                g, up = SWEEPS[s]
                nw = len(up)
                if g < 2:
                    kt_r.append(kt_sb[s].rearrange("p (c w l) -> p c w l", c=CB, w=nw))
                    qt_r.append(qt_sb[s].rearrange("p (i w l) -> p i w l", i=4, w=nw))
                else:
                    kt_r.append(kt_sb[s].rearrange("p (h c l) -> p h c l", h=2, c=CB))
                    qt_r.append(qt_sb[s].rearrange("p (h i l) -> p h i l", h=2, i=4))
                v_r.append(v_sb[s].rearrange("p (h c w x) -> p h c w x",
                                             h=2, c=CB, w=nw))
                out_dr.append(out_d[s].rearrange("p (h i w d x) -> p h i w d x",
                                                 h=2, i=4, w=nw, d=5))

            # ---- sweeps, interleaved in pairs; AV delayed one cb-beat so it
            # reads last beat's exp while this beat's exp streams on ACT
            unit_idx = 0
            at_tiles = {}

            def emit_scores(s, ci):
                g, up = SWEEPS[s]
                nw = len(up)
                sp = ps_s.tile([128, 2, 2, 256], f32, tag="s", name="sp")
                off = 0
                for (cb, i0, i1) in SCHUNKS[ci]:
                    nr = i1 - i0 + 1
                    for h in range(2):
                        for us, u in enumerate(up):
                            if g < 2:
                                lhsT = kt_r[s][64 * h:64 * h + 64, cb, us, :]
                                rhs = qt_r[s][64 * h:64 * h + 64,
                                              i0:i1 + 1, us, :]
                            else:
                                lhsT = kt_r[s][0:69, h, cb, :]
                                rhs = qt_r[s][0:69, h, i0:i1 + 1, :]
                            nc.tensor.matmul(
                                sp[:, h, us, off:off + nr * 128], lhsT, rhs,
                                start=True, stop=True)
                    off += nr * 128
                at = atp.tile([128, 2, 2, 256], adt, tag="at", name="at")
                nc.scalar.activation(at[:, :, 0:nw, 0:off],
                                     sp[:, :, 0:nw, 0:off],
                                     exp_fn, scale=SCALE,
                                     bias=AT_BIAS if AT_DTYPE == "f8e4" else 0.0)
                at_tiles[(s, ci)] = at

            def emit_av(s, i):
                nonlocal unit_idx
                g, up = SWEEPS[s]
                nw = len(up)
                for h in range(2):
                    po = ps_o.tile([128, 2, 512], f32, tag="o", name="po")
                    for d in range(5):
                        ccb = i + d
                        ci, aoff = AVMAP[(i, ccb)]
                        a_t = at_tiles[(s, ci)]
                        for us, u in enumerate(up):
                            lhsT = a_t[:, h, us, aoff:aoff + 128]
                            rhs = v_r[s][:, h, ccb, us, 0:65]
                            nc.tensor.matmul(po[:, us, d * 66:d * 66 + 65],
                                             lhsT, rhs, start=True, stop=True)
                    pv = po[:, :, 0:330].rearrange("p u (d c) -> p u d c", c=66)
                    rc = small.tile([128, 2, 5], f32, tag="rc", name="rc")
                    nc.vector.reciprocal_approx_fast(
                        out=rc[:, 0:nw], in_=pv[:, 0:nw, 0:5, 64])
                    tmp = small.tile([128, 2, 5, 64], odt, tag="tmp",
                                     name="tmp")
                    nc.vector.tensor_mul(
                        tmp[:, 0:nw], pv[:, 0:nw, 0:5, 0:64],
                        rc[:, 0:nw].broadcast_to([128, nw, 5, 64]))
                    # ship per-delta products; host sums over d
                    queues[unit_idx % nq].dma_start(
                        out=out_dr[s][:, h, i], in_=tmp[:, 0:nw])
                    unit_idx += 1

            nchunk = len(SCHUNKS)
            for pair in SWEEP_PAIRS:
                for t in range(nchunk + 1):
                    for s in pair:
                        if t < nchunk:
                            emit_scores(s, t)
                        for i in FIRE.get(t, []):
                            emit_av(s, i)

    nc.compile()
    _BASS_CACHE["nc"] = nc
    return nc


# ---------------------------------------------------------------- host pack
def _np_dtype(name):
    if name == "bf16":
        import ml_dtypes
        return ml_dtypes.bfloat16
    return np.float16


def _pack_inputs(q, k, v):
    q = np.asarray(q, dtype=np.float32)
    k = np.asarray(k, dtype=np.float32)
    v = np.asarray(v, dtype=np.float32)
    s_np = _np_dtype(S_DTYPE)
    v_np = _np_dtype(V_DTYPE)
    qb = q.reshape(NB, BLOCK, H, D)
    kb = k.reshape(NB, BLOCK, H, D)
    vb = v.reshape(NB, BLOCK, H, D)

    sub = np.repeat(np.arange(4), 32)
    U = (sub[None, :] == np.arange(4)[:, None]).astype(np.float32)  # [4,128]

    in_maps = []
    for core in range(NCORES):
        hh, R = core // 4, core % 4
        m = {}
        for s in range(4):
            g, upair = SWEEPS[s]
            nw = len(upair)
            pos = POS[g]
            if g < 2:
                kt = np.zeros((128, CB, nw, 128), np.float32)
                qt = np.zeros((128, 4, nw, 128), np.float32)
            else:
                kt = np.zeros((69, 2, CB, 128), np.float32)
                qt = np.zeros((69, 2, 4, 128), np.float32)
            va = np.zeros((128, 2, CB, nw, 66), np.float32)
            for h in range(2):
                head = 4 * g + 2 * hh + h
                for i in range(ROWS_PER_CORE):
                    r = 4 * R + i
                    for us, u in enumerate(upair):
                        tok = qb[r, pos[u * 128:(u + 1) * 128], head, :]  # [128,64]
                        if g < 2:
                            qt[64 * h:64 * h + 64, i, us] = tok.T
                        else:
                            qt[0:64, h, i] = tok.T
                            qt[64, h, i] = -MASK_M
                            qt[65:69, h, i] = MASK_M * U
                for cb in range(CB):
                    c = 4 * R - 2 + cb
                    valid = 0 <= c < NB
                    for us, u in enumerate(upair):
                        if valid:
                            tok = kb[c, pos[u * 128:(u + 1) * 128], head, :]
                            vt = vb[c, pos[u * 128:(u + 1) * 128], head, :]
                        else:
                            tok = np.zeros((128, D), np.float32)
                            vt = np.zeros((128, D), np.float32)
                        if g < 2:
                            kt[64 * h:64 * h + 64, cb, us] = tok.T
                        else:
                            kt[0:64, h, cb] = tok.T
                            if valid:
                                kt[64, h, cb] = 1.0
                                kt[65:69, h, cb] = U
                        va[:, h, cb, us, 0:64] = vt
                        va[:, h, cb, us, 64] = 1.0
            m[f"kt{s}"] = kt.reshape(_shapes(s)[0]).astype(s_np)
            m[f"qt{s}"] = qt.reshape(_shapes(s)[1]).astype(s_np)
            m[f"v{s}"] = va.reshape(_shapes(s)[2]).astype(v_np)
        in_maps.append(m)
    return in_maps


def _unpack(results):
    out = np.zeros((B, S, H, D), np.float32)
    for core in range(NCORES):
        hh, R = core // 4, core % 4
        res = results[core]
        for s in range(4):
            g, upair = SWEEPS[s]
            nw = len(upair)
            pos = POS[g]
            og = np.asarray(res[f"out{s}"], dtype=np.float32)
            og = og.reshape(128, 2, 4, nw, 5, 64).sum(axis=4)
            for h in range(2):
                head = 4 * g + 2 * hh + h
                for i in range(ROWS_PER_CORE):
                    r = 4 * R + i
                    for us, u in enumerate(upair):
                        out[0, r * 512 + pos[u * 128:(u + 1) * 128], head, :] = \
                            og[:, h, i, us]
    return out


# ---------------------------------------------------------------- entry
def _run(q, k, v, trace=False):
    from concourse.bass_utils import run_bass_kernel_spmd
    nc = _build_bass()
    in_maps = _pack_inputs(q, k, v)
    res = run_bass_kernel_spmd(nc, in_maps, core_ids=list(range(NCORES)),
                               trace=trace)
    return _unpack(res.results), res


def kernel(q, k, v):
    out, _ = _run(q, k, v, trace=False)
    return out
